# revision 12
# baseline (speedup 1.0000x reference)
"""Trainium2 Bass kernel: 4-layer sigmoid autoencoder forward + per-sample Jacobian.

Reference computes, per sample b:
    c1 = sig(x W1^T + b1); c2 = sig(c1 W2^T + b2); c3 = sig(c2 W3^T + b3)
    recover = c3 W4^T + b4
    Jac_b = W4 diag(s3_b) W3 diag(s2_b) W2 diag(s1_b) W1      (s = c(1-c))

Key algebraic restructure: factor through the H2=128 bottleneck:
    LT_b = (diag(s3_b) W3)^T W4^T          [H2, D]
    R_b  = diag(s2_b) W2 diag(s1_b) W1     [H2, D]
    Jac_b = LT_b^T @ R_b                   rank-128 product, 268M MACs/sample
vs the reference einsum chain's 671M MACs/sample.

Distribution: pure data parallel over batch. 8 cores x 16 samples each.
Weights replicated; all transposed layouts precomputed on host. The big
matmuls run as float32r (full PE rate on TRN2).

Hardware constraint shaping the code: a self-loading f32/f32r matmul has a
single sync-wait slot in its lowered LDW struct, so every matmul may depend
on at most ONE not-yet-observed proc. Hence:
  - all f32 inputs arrive in ONE mega-DMA ("wall", one HW queue) and the
    f32r inputs in ONE casting gpsimd DMA ("wr", one SW queue);
  - three dummy matmuls make PE observe those queues (and gpsimd) up front;
  - PSUM pools are tag-split so a tile slot is only ever *read* by one
    engine class, letting the WAR wait merge with the RAW wait.
"""

import numpy as np

import concourse.bass as bass
import concourse.mybir as mybir
import concourse.tile as tile
from concourse import bacc
from concourse.bass_utils import run_bass_kernel_spmd
from concourse.masks import make_identity

B, D, H1, H2 = 128, 1024, 512, 128
NCORES = 8
BS = B // NCORES  # 16 samples per core

F32 = mybir.dt.float32
F32R = mybir.dt.float32r
AF = mybir.ActivationFunctionType
ALU = mybir.AluOpType

# wall (f32) per-partition element offsets
O_W1T = 0          # [128, 8, 512]
O_W2T = 4096       # [128, 4, 128]
O_W3R = 4608       # [128, 4, 128]
O_W3T = 5120       # [128, 512]
O_W4T = 5632       # [128, 4, 1024]
O_XC = 9728        # [128, 8, 16]
O_B1 = 9856        # [128, 4]
O_B2 = 9860        # [128, 1]
O_B3 = 9861        # [128, 4]
O_B4 = 9865        # [128, 8]
WALL_F = 9873
WR_F = 8192        # w1r [128,4,1024] | w4tr [128,4,1024]


def _p(a, pin=128):
    """[K*pin, F...] -> [pin, K, F...] partition-major layout, contiguous."""
    a = np.ascontiguousarray(a)
    k = a.shape[0] // pin
    return np.ascontiguousarray(
        a.reshape(k, pin, *a.shape[1:]).transpose(1, 0, *range(2, a.ndim + 1))
    )


def build_nc():
    nc = bacc.Bacc()

    wall_e = nc.declare_dram_parameter("wall", [128, WALL_F], F32, isOutput=False)
    wr_e = nc.declare_dram_parameter("wr", [128, WR_F], F32, isOutput=False)
    rec_e = nc.declare_dram_parameter("recover", [BS, D], F32, isOutput=True)
    c2_e = nc.declare_dram_parameter("c2out", [BS, H2], F32, isOutput=True)
    jac_e = nc.declare_dram_parameter("jac", [BS, D, D], F32, isOutput=True)

    with tile.TileContext(nc) as tc:
        with (
            tc.tile_pool(name="w", bufs=1) as wp,
            tc.tile_pool(name="act", bufs=1) as ap,
            tc.tile_pool(name="samp", bufs=2) as sp,
            tc.tile_pool(name="jout", bufs=6) as jp,
            tc.tile_pool(name="psf", bufs=2, space="PSUM") as psf,
            tc.tile_pool(name="pst", bufs=1, space="PSUM") as pst,
            tc.tile_pool(name="psd", bufs=1, space="PSUM") as psd,
            tc.tile_pool(name="psja", bufs=2, space="PSUM") as psja,
            tc.tile_pool(name="psjd", bufs=2, space="PSUM") as psjd,
        ):
            WALL = wp.tile([128, WALL_F], F32)
            nc.sync.dma_start(WALL[:], wall_e[:])
            WR = wp.tile([128, WR_F], F32R)
            nc.gpsimd.dma_start(WR[:], wr_e[:])
            IDN = wp.tile([128, 128], F32)
            make_identity(nc, IDN[:])

            W1T = WALL[:, O_W1T:O_W1T + 4096].rearrange("p (a b) -> p a b", b=512)
            W2T = WALL[:, O_W2T:O_W2T + 512].rearrange("p (a b) -> p a b", b=128)
            W3R = WALL[:, O_W3R:O_W3R + 512].rearrange("p (a b) -> p a b", b=128)
            W3T = WALL[:, O_W3T:O_W3T + 512]
            W4T = WALL[:, O_W4T:O_W4T + 4096].rearrange("p (a b) -> p a b", b=1024)
            XC = WALL[:, O_XC:O_XC + 128].rearrange("p (a b) -> p a b", b=BS)
            B1 = WALL[:, O_B1:O_B1 + 4]
            B2 = WALL[:, O_B2:O_B2 + 1]
            B3 = WALL[:, O_B3:O_B3 + 4]
            B4 = WALL[:, O_B4:O_B4 + 8]
            W1R = WR[:, 0:4096].rearrange("p (a b) -> p a b", b=1024)
            W4TR = WR[:, 4096:8192].rearrange("p (a b) -> p a b", b=1024)

            mm = nc.tensor.matmul

            # --- dummy ladder: PE observes wall-HW-queue, wr-SW-queue, gpsimd
            pd = psd.tile([2, 2], F32, tag="dum")
            for src in (WALL[:, 0:2], WR[:, 0:2], IDN[:, 0:2]):
                mm(pd[:], src, src, start=True, stop=True)

            # ---------------- forward pass (batched over 16 samples) ----------
            # activations kept transposed: cT[feature_part, sample]
            c1T = ap.tile([128, 4, BS], F32)
            s1T = ap.tile([128, 4, BS], F32)
            for m in range(4):
                p = psf.tile([128, BS], F32, tag="f")
                for k in range(8):
                    mm(p[:], W1T[:, k, m * 128:(m + 1) * 128], XC[:, k, :],
                       start=(k == 0), stop=(k == 7))
                nc.scalar.activation(c1T[:, m, :], p[:], AF.Sigmoid,
                                     bias=B1[:, m:m + 1])
            nc.vector.tensor_tensor(s1T[:], c1T[:], c1T[:], ALU.mult)
            nc.vector.tensor_tensor(s1T[:], c1T[:], s1T[:], ALU.subtract)

            c2T = ap.tile([128, BS], F32)
            s2T = ap.tile([128, BS], F32)
            p = psf.tile([128, BS], F32, tag="f")
            for k in range(4):
                mm(p[:], W2T[:, k, :], c1T[:, k, :], start=(k == 0), stop=(k == 3))
            nc.scalar.activation(c2T[:], p[:], AF.Sigmoid, bias=B2[:, 0:1])
            nc.vector.tensor_tensor(s2T[:], c2T[:], c2T[:], ALU.mult)
            nc.vector.tensor_tensor(s2T[:], c2T[:], s2T[:], ALU.subtract)

            # c2 output [BS, H2] via PE transpose
            tp = pst.tile([BS, 128], F32, tag="pst")
            nc.tensor.transpose(tp[:], c2T[:], IDN[:])
            c2sb = ap.tile([BS, 128], F32)
            nc.scalar.copy(c2sb[:], tp[:])
            nc.sync.dma_start(c2_e[:], c2sb[:])

            c3T = ap.tile([128, 4, BS], F32)
            s3T = ap.tile([128, 4, BS], F32)
            for m in range(4):
                p = psf.tile([128, BS], F32, tag="f")
                mm(p[:], W3T[:, m * 128:(m + 1) * 128], c2T[:], start=True,
                   stop=True)
                nc.scalar.activation(c3T[:, m, :], p[:], AF.Sigmoid,
                                     bias=B3[:, m:m + 1])
            nc.vector.tensor_tensor(s3T[:], c3T[:], c3T[:], ALU.mult)
            nc.vector.tensor_tensor(s3T[:], c3T[:], s3T[:], ALU.subtract)

            # recover [BS, D] = c3 W4^T + b4
            recsb = ap.tile([BS, D], F32)
            for m in range(8):
                p = psf.tile([128, BS], F32, tag="f")
                for k in range(4):
                    mm(p[:], W4T[:, k, m * 128:(m + 1) * 128], c3T[:, k, :],
                       start=(k == 0), stop=(k == 3))
                rts = ap.tile([128, BS], F32, tag="rts")
                nc.scalar.activation(rts[:], p[:], AF.Identity, bias=B4[:, m:m + 1])
                tp = pst.tile([BS, 128], F32, tag="pst")
                nc.tensor.transpose(tp[:], rts[:], IDN[:])
                nc.scalar.copy(recsb[:, m * 128:(m + 1) * 128], tp[:])
            nc.sync.dma_start(rec_e[:], recsb[:])

            # ---------------- Jacobian (per sample) ---------------------------
            # ACT-copied jac tiles come first so PE observes the ACT tick for
            # Rs/Ls via a single wait; the DVE-copied ones then ride on DVE.
            ACT_TILES = 6
            for b in range(BS):
                w2s = sp.tile([128, 4, H2], F32R, tag="w2s")
                nc.vector.tensor_tensor(
                    w2s[:], W2T[:],
                    s1T[:, :, b:b + 1].to_broadcast([128, 4, H2]), ALU.mult)
                w3s = sp.tile([128, 4, H2], F32R, tag="w3s")
                nc.vector.tensor_tensor(
                    w3s[:], W3R[:],
                    s3T[:, :, b:b + 1].to_broadcast([128, 4, H2]), ALU.mult)

                Rs = sp.tile([128, D], F32R, tag="rs")
                Ls = sp.tile([128, D], F32R, tag="ls")
                for n in range(2):
                    nsl = slice(n * 512, (n + 1) * 512)
                    rp = psf.tile([128, 512], F32, tag="f")
                    for k in range(4):
                        mm(rp[:], w2s[:, k, :], W1R[:, k, nsl],
                           start=(k == 0), stop=(k == 3))
                    # fold the diag(s2) row-scale into the PSUM->SBUF copy
                    nc.scalar.activation(Rs[:, nsl], rp[:], AF.Copy,
                                         scale=s2T[:, b:b + 1])
                    lp = psf.tile([128, 512], F32, tag="f")
                    for k in range(4):
                        mm(lp[:], w3s[:, k, :], W4TR[:, k, nsl],
                           start=(k == 0), stop=(k == 3))
                    nc.scalar.copy(Ls[:, nsl], lp[:])

                for t in range(16):
                    m, n = t // 2, t % 2
                    nsl = slice(n * 512, (n + 1) * 512)
                    on_act = t < ACT_TILES
                    pool = psja if on_act else psjd
                    jpx = pool.tile([128, 512], F32, tag="ja" if on_act else "jd")
                    mm(jpx[:], Ls[:, m * 128:(m + 1) * 128], Rs[:, nsl],
                       start=True, stop=True)
                    jsb = jp.tile([128, 512], F32, tag="jsb")
                    if on_act:
                        nc.scalar.copy(jsb[:], jpx[:])
                    else:
                        nc.vector.tensor_copy(jsb[:], jpx[:])
                    nc.sync.dma_start(
                        jac_e[b, m * 128:(m + 1) * 128, nsl], jsb[:])

    return nc


def _prep_wall(W1, b1, W2, b2, W3, b3, W4, b4):
    """Per-core-shared [128, WALL_F] f32 block and [128, WR_F] f32r block."""
    f = np.float32
    parts = [
        _p(np.ascontiguousarray(W1.T)).reshape(128, -1),   # w1t [128, 4096]
        _p(np.ascontiguousarray(W2.T)).reshape(128, -1),   # w2t [128, 512]
        _p(W3).reshape(128, -1),                           # w3r [128, 512]
        np.ascontiguousarray(W3.T),                        # w3t [128, 512]
        _p(np.ascontiguousarray(W4.T)).reshape(128, -1),   # w4t [128, 4096]
        np.zeros((128, 128), f),                           # xc placeholder
        np.ascontiguousarray(b1.reshape(4, 128).T),
        np.ascontiguousarray(b2.reshape(128, 1)),
        np.ascontiguousarray(b3.reshape(4, 128).T),
        np.ascontiguousarray(b4.reshape(8, 128).T),
    ]
    wall = np.concatenate(parts, axis=1).astype(f)
    assert wall.shape == (128, WALL_F), wall.shape
    wr = np.concatenate(
        [_p(W1).reshape(128, -1),
         _p(np.ascontiguousarray(W4.T)).reshape(128, -1)], axis=1).astype(f)
    assert wr.shape == (128, WR_F), wr.shape
    return wall, wr


_CACHE = {}


def _get_nc():
    if "nc" not in _CACHE:
        nc = build_nc()
        if not nc.is_finalized():
            nc.finalize()
        _CACHE["nc"] = nc
    return _CACHE["nc"]


def run(x, W1, b1, W2, b2, W3, b3, W4, b4, trace=False, **spmd_kwargs):
    f = lambda a: np.ascontiguousarray(np.asarray(a, dtype=np.float32))
    x, W1, b1, W2, b2, W3, b3, W4, b4 = map(
        f, (x, W1, b1, W2, b2, W3, b3, W4, b4))
    wall, wr = _prep_wall(W1, b1, W2, b2, W3, b3, W4, b4)
    in_maps = []
    for i in range(NCORES):
        xs = x[i * BS:(i + 1) * BS]          # [16, 1024]
        xc = _p(np.ascontiguousarray(xs.T)).reshape(128, -1)  # [128, 128]
        w = wall.copy()
        w[:, O_XC:O_XC + 128] = xc
        in_maps.append({"wall": w, "wr": wr})
    res = run_bass_kernel_spmd(
        _get_nc(), in_maps, core_ids=list(range(NCORES)), trace=trace,
        **spmd_kwargs)
    recover = np.concatenate([r["recover"] for r in res.results], axis=0)
    c2 = np.concatenate([r["c2out"] for r in res.results], axis=0)
    jac = np.concatenate([r["jac"] for r in res.results], axis=0)
    return (recover, c2, jac), res


def kernel(x, W1, b1, W2, b2, W3, b3, W4, b4):
    out, _ = run(x, W1, b1, W2, b2, W3, b3, W4, b4)
    return out


# revision 14
# speedup vs baseline: 1.2599x; 1.2599x over previous
"""Trainium2 Bass kernel: 4-layer sigmoid autoencoder forward + per-sample Jacobian.

Reference computes, per sample b:
    c1 = sig(x W1^T + b1); c2 = sig(c1 W2^T + b2); c3 = sig(c2 W3^T + b3)
    recover = c3 W4^T + b4
    Jac_b = W4 diag(s3_b) W3 diag(s2_b) W2 diag(s1_b) W1      (s = c(1-c))

Key algebraic restructure: factor through the H2=128 bottleneck:
    LT_b = (diag(s3_b) W3)^T W4^T          [H2, D]
    R_b  = diag(s2_b) W2 diag(s1_b) W1     [H2, D]
    Jac_b = LT_b^T @ R_b                   rank-128 product, 268M MACs/sample
vs the reference einsum chain's 671M MACs/sample.

Distribution: pure data parallel over batch. 8 cores x 16 samples each.
Weights replicated; all transposed layouts precomputed on host. The
Jacobian-path matmuls run in bf16 (f32 PSUM accumulate, f32 output); the
forward pass stays f32 for the recover/c2 outputs.

Shape of the implementation, driven by what the trace showed:
  - a self-loading f32 matmul has ONE sync-wait slot, and every extra wait
    becomes an event-semaphore chain after bacc lowering — so inputs arrive
    in few mega-DMAs, a dummy-matmul ladder makes PE observe each input
    queue once, and PSUM slots are tag-split so each slot is only read by
    one engine class (WAR merges with RAW into a single wait);
  - PE warmup matmuls run under the input DMAs so the HAM clock gate is
    released before real work starts, and the R/L matmuls of sample b+1 are
    emitted BEFORE the jac tiles of sample b (software pipelining) so the
    in-order PE never idles long enough to re-throttle;
  - jac tiles are 2-PSUM-bank [128, 1024] blocks: one copy instruction and
    one fully-contiguous 512KB DMA each.
"""

import numpy as np
import ml_dtypes

import concourse.bass as bass
import concourse.mybir as mybir
import concourse.tile as tile
from concourse import bacc
from concourse.bass_utils import run_bass_kernel_spmd
from concourse.masks import make_identity

B, D, H1, H2 = 128, 1024, 512, 128
NCORES = 8
BS = B // NCORES  # 16 samples per core

F32 = mybir.dt.float32
BF16 = mybir.dt.bfloat16
AF = mybir.ActivationFunctionType
ALU = mybir.AluOpType

# wall_a (f32): forward layer-1 critical inputs
A_W1T = 0          # [128, 8, 512]
A_XC = 4096        # [128, 8, 16]
A_B1 = 4224        # [128, 4]
WA_F = 4228
# wall_b (f32): rest of forward weights/biases
B_W2T = 0          # [128, 4, 128]
B_W3R = 512        # [128, 4, 128]
B_W3T = 1024       # [128, 512]
B_W4T = 1536       # [128, 4, 1024]
B_B2 = 5632        # [128, 1]
B_B3 = 5633        # [128, 4]
B_B4 = 5637        # [128, 8]
WB_F = 5645
WR_F = 8192        # bf16: w1r [128,4,1024] | w4tr [128,4,1024]


def _p(a, pin=128):
    """[K*pin, F...] -> [pin, K, F...] partition-major layout, contiguous."""
    a = np.ascontiguousarray(a)
    k = a.shape[0] // pin
    return np.ascontiguousarray(
        a.reshape(k, pin, *a.shape[1:]).transpose(1, 0, *range(2, a.ndim + 1))
    )


def build_nc():
    nc = bacc.Bacc()

    wa_e = nc.declare_dram_parameter("wall_a", [128, WA_F], F32, isOutput=False)
    wb_e = nc.declare_dram_parameter("wall_b", [128, WB_F], F32, isOutput=False)
    wr_e = nc.declare_dram_parameter("wr", [128, WR_F], BF16, isOutput=False)
    rec_e = nc.declare_dram_parameter("recover", [BS, D], F32, isOutput=True)
    c2_e = nc.declare_dram_parameter("c2out", [BS, H2], F32, isOutput=True)
    jac_e = nc.declare_dram_parameter("jac", [BS, D, D], F32, isOutput=True)

    with tile.TileContext(nc) as tc:
        with (
            tc.tile_pool(name="w", bufs=1) as wp,
            tc.tile_pool(name="act", bufs=1) as ap,
            tc.tile_pool(name="samp", bufs=2) as sp,
            tc.tile_pool(name="jout", bufs=4) as jp,
            tc.tile_pool(name="psA", bufs=2, space="PSUM") as psf,
            tc.tile_pool(name="psja", bufs=1, space="PSUM") as psja,
            tc.tile_pool(name="psjd", bufs=1, space="PSUM") as psjd,
        ):
            IDN = wp.tile([128, 128], F32)
            make_identity(nc, IDN[:])
            WA = wp.tile([128, WA_F], F32)
            nc.sync.dma_start(WA[:], wa_e[:])
            WB = wp.tile([128, WB_F], F32)
            nc.sync.dma_start(WB[:], wb_e[:])
            WR = wp.tile([128, WR_F], BF16)
            nc.sync.dma_start(WR[:], wr_e[:])

            W1T = WA[:, A_W1T:A_W1T + 4096].rearrange("p (a b) -> p a b", b=512)
            XC = WA[:, A_XC:A_XC + 128].rearrange("p (a b) -> p a b", b=BS)
            B1 = WA[:, A_B1:A_B1 + 4]
            W2T = WB[:, B_W2T:B_W2T + 512].rearrange("p (a b) -> p a b", b=128)
            W3R = WB[:, B_W3R:B_W3R + 512].rearrange("p (a b) -> p a b", b=128)
            W3T = WB[:, B_W3T:B_W3T + 512]
            W4T = WB[:, B_W4T:B_W4T + 4096].rearrange("p (a b) -> p a b", b=1024)
            B2 = WB[:, B_B2:B_B2 + 1]
            B3 = WB[:, B_B3:B_B3 + 4]
            B4 = WB[:, B_B4:B_B4 + 8]
            W1R = WR[:, 0:4096].rearrange("p (a b) -> p a b", b=1024)
            W4TR = WR[:, 4096:8192].rearrange("p (a b) -> p a b", b=1024)

            mm = nc.tensor.matmul

            # --- PE warmup under the input DMAs: releases the HAM clock
            # gate (~3.4us of sustained PE activity) before real work.
            pw = psf.tile([128, 128], F32, tag="f")
            for _ in range(28):
                mm(pw[:], IDN[:], IDN[:], start=True, stop=True)
            # --- dummy ladder: PE observes each input DMA queue once.
            pd = psf.tile([2, 2], F32, tag="f")
            for src in (WA[:, 0:2], WB[:, 0:2], WR[:, 0:2]):
                mm(pd[:], src, src, start=True, stop=True)

            # ---------------- forward pass (batched over 16 samples) ----------
            # activations kept transposed: cT[feature_part, sample]
            c1T = ap.tile([128, 4, BS], F32)
            s1T = ap.tile([128, 4, BS], F32)
            for m in range(4):
                p = psf.tile([128, BS], F32, tag="f")
                for k in range(8):
                    mm(p[:], W1T[:, k, m * 128:(m + 1) * 128], XC[:, k, :],
                       start=(k == 0), stop=(k == 7))
                nc.scalar.activation(c1T[:, m, :], p[:], AF.Sigmoid,
                                     bias=B1[:, m:m + 1])
            nc.vector.tensor_tensor(s1T[:], c1T[:], c1T[:], ALU.mult)
            nc.vector.tensor_tensor(s1T[:], c1T[:], s1T[:], ALU.subtract)

            c2T = ap.tile([128, BS], F32)
            s2T = ap.tile([128, BS], F32)
            p = psf.tile([128, BS], F32, tag="f")
            for k in range(4):
                mm(p[:], W2T[:, k, :], c1T[:, k, :], start=(k == 0), stop=(k == 3))
            nc.scalar.activation(c2T[:], p[:], AF.Sigmoid, bias=B2[:, 0:1])
            nc.vector.tensor_tensor(s2T[:], c2T[:], c2T[:], ALU.mult)
            nc.vector.tensor_tensor(s2T[:], c2T[:], s2T[:], ALU.subtract)

            # c2 output [BS, H2] via PE transpose
            tp = psf.tile([BS, 128], F32, tag="f")
            nc.tensor.transpose(tp[:], c2T[:], IDN[:])
            c2sb = ap.tile([BS, 128], F32)
            nc.scalar.copy(c2sb[:], tp[:])
            nc.sync.dma_start(c2_e[:], c2sb[:])

            c3T = ap.tile([128, 4, BS], F32)
            s3T = ap.tile([128, 4, BS], F32)
            for m in range(4):
                p = psf.tile([128, BS], F32, tag="f")
                mm(p[:], W3T[:, m * 128:(m + 1) * 128], c2T[:], start=True,
                   stop=True)
                nc.scalar.activation(c3T[:, m, :], p[:], AF.Sigmoid,
                                     bias=B3[:, m:m + 1])
            nc.vector.tensor_tensor(s3T[:], c3T[:], c3T[:], ALU.mult)
            nc.vector.tensor_tensor(s3T[:], c3T[:], s3T[:], ALU.subtract)

            # recover [BS, D] = c3 W4^T + b4
            recsb = ap.tile([BS, D], F32)
            for m in range(8):
                p = psf.tile([128, BS], F32, tag="f")
                for k in range(4):
                    mm(p[:], W4T[:, k, m * 128:(m + 1) * 128], c3T[:, k, :],
                       start=(k == 0), stop=(k == 3))
                rts = ap.tile([128, BS], F32, tag="rts")
                nc.scalar.activation(rts[:], p[:], AF.Identity, bias=B4[:, m:m + 1])
                tp = psf.tile([BS, 128], F32, tag="f")
                nc.tensor.transpose(tp[:], rts[:], IDN[:])
                nc.scalar.copy(recsb[:, m * 128:(m + 1) * 128], tp[:])
            nc.sync.dma_start(rec_e[:], recsb[:])

            # ---------------- Jacobian (software-pipelined over samples) ------
            # Per iteration: copy out sample b's R/L factors, then emit the
            # R/L matmuls of sample b+1 (so the in-order PE has dense work
            # while b's jac tiles wait on copy/DMA slots), then b's jac tiles.
            def scale_ws(b):
                w2s = sp.tile([128, 4, H2], BF16, tag="w2s")
                nc.gpsimd.tensor_tensor(
                    w2s[:], W2T[:],
                    s1T[:, :, b:b + 1].to_broadcast([128, 4, H2]), ALU.mult)
                w3s = sp.tile([128, 4, H2], BF16, tag="w3s")
                nc.gpsimd.tensor_tensor(
                    w3s[:], W3R[:],
                    s3T[:, :, b:b + 1].to_broadcast([128, 4, H2]), ALU.mult)
                return w2s, w3s

            def rl_mms(w2s, w3s):
                rp = psf.tile([128, D], F32, tag="f")
                lp = psf.tile([128, D], F32, tag="f")
                for n in range(2):
                    nsl = slice(n * 512, (n + 1) * 512)
                    for k in range(4):
                        mm(rp[:, nsl], w2s[:, k, :], W1R[:, k, nsl],
                           start=(k == 0), stop=(k == 3))
                    for k in range(4):
                        mm(lp[:, nsl], w3s[:, k, :], W4TR[:, k, nsl],
                           start=(k == 0), stop=(k == 3))
                return rp, lp

            rp, lp = rl_mms(*scale_ws(0))
            for b in range(BS):
                Rs = sp.tile([128, D], BF16, tag="rs")
                Ls = sp.tile([128, D], BF16, tag="ls")
                # fold the diag(s2) row-scale into the PSUM->SBUF copy
                nc.scalar.activation(Rs[:], rp[:], AF.Copy,
                                     scale=s2T[:, b:b + 1])
                nc.scalar.copy(Ls[:], lp[:])
                if b + 1 < BS:
                    rp, lp = rl_mms(*scale_ws(b + 1))

                for m in range(8):
                    on_act = m in (0, 3, 6)
                    pool = psja if on_act else psjd
                    jpx = pool.tile([128, D], F32, tag="ja" if on_act else "jd")
                    for n in range(2):
                        nsl = slice(n * 512, (n + 1) * 512)
                        mm(jpx[:, nsl], Ls[:, m * 128:(m + 1) * 128],
                           Rs[:, nsl], start=True, stop=True)
                    jsb = jp.tile([128, D], F32, tag="jsb")
                    if on_act:
                        nc.scalar.copy(jsb[:], jpx[:])
                    else:
                        nc.vector.tensor_copy(jsb[:], jpx[:])
                    nc.sync.dma_start(jac_e[b, m * 128:(m + 1) * 128, :], jsb[:])

    return nc


def _prep_walls(W1, b1, W2, b2, W3, b3, W4, b4):
    """Host-packed input blocks shared by all cores (minus the xc slot)."""
    f = np.float32
    w4t = _p(np.ascontiguousarray(W4.T)).reshape(128, -1)
    wall_a = np.concatenate([
        _p(np.ascontiguousarray(W1.T)).reshape(128, -1),   # w1t [128, 4096]
        np.zeros((128, 128), f),                           # xc placeholder
        np.ascontiguousarray(b1.reshape(4, 128).T),
    ], axis=1).astype(f)
    assert wall_a.shape == (128, WA_F), wall_a.shape
    wall_b = np.concatenate([
        _p(np.ascontiguousarray(W2.T)).reshape(128, -1),   # w2t [128, 512]
        _p(W3).reshape(128, -1),                           # w3r [128, 512]
        np.ascontiguousarray(W3.T),                        # w3t [128, 512]
        w4t,                                               # w4t [128, 4096]
        np.ascontiguousarray(b2.reshape(128, 1)),
        np.ascontiguousarray(b3.reshape(4, 128).T),
        np.ascontiguousarray(b4.reshape(8, 128).T),
    ], axis=1).astype(f)
    assert wall_b.shape == (128, WB_F), wall_b.shape
    wr = np.concatenate(
        [_p(W1).reshape(128, -1), w4t], axis=1).astype(ml_dtypes.bfloat16)
    assert wr.shape == (128, WR_F), wr.shape
    return wall_a, wall_b, wr


_CACHE = {}


def _get_nc():
    if "nc" not in _CACHE:
        nc = build_nc()
        if not nc.is_finalized():
            nc.finalize()
        _CACHE["nc"] = nc
    return _CACHE["nc"]


def run(x, W1, b1, W2, b2, W3, b3, W4, b4, trace=False, **spmd_kwargs):
    f = lambda a: np.ascontiguousarray(np.asarray(a, dtype=np.float32))
    x, W1, b1, W2, b2, W3, b3, W4, b4 = map(
        f, (x, W1, b1, W2, b2, W3, b3, W4, b4))
    wall_a, wall_b, wr = _prep_walls(W1, b1, W2, b2, W3, b3, W4, b4)
    in_maps = []
    for i in range(NCORES):
        xs = x[i * BS:(i + 1) * BS]          # [16, 1024]
        xc = _p(np.ascontiguousarray(xs.T)).reshape(128, -1)  # [128, 128]
        wa = wall_a.copy()
        wa[:, A_XC:A_XC + 128] = xc
        in_maps.append({"wall_a": wa, "wall_b": wall_b, "wr": wr})
    res = run_bass_kernel_spmd(
        _get_nc(), in_maps, core_ids=list(range(NCORES)), trace=trace,
        **spmd_kwargs)
    recover = np.concatenate([r["recover"] for r in res.results], axis=0)
    c2 = np.concatenate([r["c2out"] for r in res.results], axis=0)
    jac = np.concatenate([r["jac"] for r in res.results], axis=0)
    return (recover, c2, jac), res


def kernel(x, W1, b1, W2, b2, W3, b3, W4, b4):
    out, _ = run(x, W1, b1, W2, b2, W3, b3, W4, b4)
    return out
